# revision 1
# baseline (speedup 1.0000x reference)
"""Trainium2 Bass kernel for nn_Attention_8839042695176.

Full (unsharded) inputs in, full output out. Internally: 8 NeuronCores,
core h owns attention head h (both batch elements), convs/qkv replicated
per core on that core's permuted channel order.

Math per (b, h) unit:
    scores[i,j] = q_full[c,i]·emb[c,j] + qd_up[c,i]·kd_up[c,j]   (K=16 matmul)
    attn = softmax_j(scores)        (no max-subtraction; |scores| <~ 8)
    out[c,i]  = sum_j attn[i,j] vv[c,j]
computed in transposed layout E^T[j,i] so both big matmuls stream on PE,
with the softmax denominator fused in as an extra all-ones row of vv^T.
"""

import numpy as np

HEADS = 8
DIM_HEAD = 8
B = 2
C = 64
H = 48
HW = H * H          # 2304
KS = 11             # conv kernel
STRIDE = 8
PAD = 2
M6 = 6              # downsampled side
MM = M6 * M6        # 36
PADW = H + 2 * PAD  # 52
PADHW = PADW * PADW # 2704
SCALE = DIM_HEAD ** (-0.5)
NCORES = 8
TAPS = KS * KS      # 121

# i-chunks for the main loop (<=512 fp32 moving limit)
ICHUNKS = [(0, 512), (512, 512), (1024, 512), (1536, 512), (2048, 256)]
# chunks for the qkv projection, row-aligned to 48 (10 rows / 8 rows)
QCHUNKS = [(0, 480), (480, 480), (960, 480), (1440, 480), (1920, 384)]
NJT = HW // 128     # 18 j-tiles

_PROGRAMS = {}
# Conv activation selector: "Gelu" on hardware; CoreSim lacks Gelu, so the
# sim test swaps in "Tanh" (np reference adapted identically).
_CONV_ACT = "Gelu"
# When True, adds debug DRAM outputs for intermediates (sim debugging only).
_DEBUG = False


def _build_program(repeat=1, split=True):
    from contextlib import ExitStack
    import concourse.bass as bass
    import concourse.mybir as mybir
    import concourse.tile as tile
    from concourse.masks import make_identity

    F32 = mybir.dt.float32
    BF = mybir.dt.bfloat16
    AF = mybir.ActivationFunctionType

    nc = bass.Bass(trn_type="TRN2")

    f2 = nc.declare_dram_parameter("f2", [B, C, HW], BF, isOutput=False)
    w1T = nc.declare_dram_parameter("w1T", [C, 128], BF, isOutput=False)
    wvT = nc.declare_dram_parameter("wvT", [C, 8], BF, isOutput=False)
    wqT = nc.declare_dram_parameter("wqT", [128, 66 * C], BF, isOutput=False)
    wkT = nc.declare_dram_parameter("wkT", [128, 66 * C], BF, isOutput=False)
    bq2 = nc.declare_dram_parameter("bq2", [C], F32, isOutput=False)
    bk2 = nc.declare_dram_parameter("bk2", [C], F32, isOutput=False)
    emb = nc.declare_dram_parameter("emb", [8, HW], BF, isOutput=False)
    out = nc.declare_dram_parameter("out", [B, 8, HW], F32, isOutput=True)
    dbg = {}
    if _DEBUG:
        for name, shape in [("d_xq", [C, B, PADHW]), ("d_xk", [C, B, PADHW]),
                            ("d_vv", [8, B, HW]), ("d_qd", [C, B, MM]),
                            ("d_kd", [C, B, MM]), ("d_s0", [16, HW]),
                            ("d_r0", [16, HW]), ("d_vt0", [128, NJT * 9])]:
            dbg[name] = nc.declare_dram_parameter(name, shape, F32, isOutput=True)

    def interior(Xt, b):
        """[64, 48, 48] strided view of the padded map's valid region."""
        return bass.AP(
            tensor=Xt.tensor,
            offset=Xt.offset + b * PADHW + PAD * PADW + PAD,
            ap=[Xt.ap[0], [PADW, H], [1, H]],
        )

    def up_ap(Dt, b, nrows=8):
        """Broadcast view: D[c, b, p] -> [c, 36(p), 64(repeat)] (flat i//64)."""
        return bass.AP(
            tensor=Dt.tensor,
            offset=Dt.offset + b * MM,
            ap=[[Dt.ap[0][0], nrows], [1, MM], [0, 64]],
        )

    with tile.TileContext(nc) as tc, ExitStack() as ctx:
        # persistent pools (whole kernel)
        const = ctx.enter_context(tc.tile_pool(name="const", bufs=1))
        work = ctx.enter_context(tc.tile_pool(name="work", bufs=3))
        epool = ctx.enter_context(tc.tile_pool(name="epool", bufs=3))

        ID8 = const.tile([8, 8], BF)
        make_identity(nc, ID8)
        ONE9 = const.tile([1, 9], F32)
        nc.vector.memset(ONE9, 1.0)

        def _rep_body():
            # ---- persistent constants ----
            W1 = const.tile([C, 128], BF)
            nc.sync.dma_start(W1, w1T[:, :])
            WV = const.tile([C, 8], BF)
            nc.sync.dma_start(WV, wvT[:, :])
            BQ = const.tile([C, 1], F32)
            nc.sync.dma_start(BQ, bq2[:].rearrange("(p f) -> p f", f=1))
            BK = const.tile([C, 1], F32)
            nc.sync.dma_start(BK, bk2[:].rearrange("(p f) -> p f", f=1))
            QD = const.tile([C, B, MM], F32)
            KD = const.tile([C, B, MM], F32)
            Ss = [const.tile([16, HW], BF, name=f"S{b}") for b in range(B)]
            Rs = [const.tile([16, HW], BF, name=f"R{b}") for b in range(B)]
            VTs = [const.tile([128, NJT, 9], BF, name=f"VT{b}") for b in range(B)]

            # ---- prologue stage (scratch released before the main loops) ----
            with tc.tile_pool(name="stage", bufs=1) as stage, \
                 tc.tile_pool(name="psum_pro", bufs=1, space="PSUM") as pp:
                F = stage.tile([C, B, HW], BF)
                for b in range(B):
                    nc.sync.dma_start(F[:, b, :], f2[b, :, :])
                XQ = stage.tile([128, B, PADHW], BF)
                XK = stage.tile([128, B, PADHW], BF)
                VV = stage.tile([8, B, HW], BF)

                # zero the padding border (3 strips per (map, batch));
                # rows 64-127 hold the map shifted left by one element, so
                # their border strips sit one element earlier.
                for Xt in (XQ, XK):
                    p0 = [Xt.ap[0][0], 64]
                    p1 = bass.AP(tensor=Xt.tensor, offset=Xt.offset + 64 * Xt.ap[0][0], ap=Xt.ap).ap
                    for b in range(B):
                        base = b * PADHW
                        for shift, poff in ((0, 0), (1, 64)):
                            o = Xt.offset + poff * Xt.ap[0][0] + base
                            pap = [[Xt.ap[0][0], 64]]
                            nc.gpsimd.memset(
                                bass.AP(tensor=Xt.tensor, offset=o,
                                        ap=pap + [[1, 2 * PADW + PAD - shift]]), 0.0)
                            nc.gpsimd.memset(
                                bass.AP(tensor=Xt.tensor,
                                        offset=o + (H + PAD - 1) * PADW + PAD + H - shift,
                                        ap=pap + [[1, 2 * PADW + PAD + shift]]), 0.0)
                            nc.gpsimd.memset(
                                bass.AP(tensor=Xt.tensor,
                                        offset=o + PAD * PADW + PAD + H - shift,
                                        ap=pap + [[PADW, H - 1], [1, 2 * PAD]]), 0.0)

                # ---- qkv projection (chunks aligned to whole 48-rows) ----
                for b in range(B):
                    for (j0, nj) in QCHUNKS:
                        nrows = nj // H
                        y0 = j0 // H
                        pq = pp.tile([128, 480], F32, tag="pq", bufs=2)
                        nc.tensor.matmul(pq[:, :nj], lhsT=W1, rhs=F[:, b, j0:j0 + nj],
                                         start=True, stop=True)
                        pv = pp.tile([8, 480], F32, tag="pv", bufs=2)
                        nc.tensor.matmul(pv[:, :nj], lhsT=WV, rhs=F[:, b, j0:j0 + nj],
                                         start=True, stop=True)
                        for Xt, r0 in ((XQ, 0), (XK, 64)):
                            src = pq[r0:r0 + 64, :nj].rearrange(
                                "p (r w) -> p r w", r=nrows, w=H)
                            for shift, poff in ((0, 0), (1, 64)):
                                dst = bass.AP(
                                    tensor=Xt.tensor,
                                    offset=(Xt.offset + poff * Xt.ap[0][0]
                                            + b * PADHW
                                            + (PAD + y0) * PADW + PAD - shift),
                                    ap=[[Xt.ap[0][0], 64], [PADW, nrows], [1, H]])
                                nc.vector.tensor_copy(dst, src)
                        nc.vector.tensor_copy(VV[:, b, j0:j0 + nj], pv[:, :nj])

                # ---- strided 11x11 convs (121 accumulated taps, both batches);
                #      WQ and WK share one weight slot sequentially ----
                for (Xt, wdram, Bt, Dt) in ((XQ, wqT, BQ, QD), (XK, wkT, BK, KD)):
                    Wc = stage.tile([128, 66 * C], BF, tag="wconv", name="Wc")
                    nc.sync.dma_start(Wc, wdram[:, :])
                    acc = pp.tile([C, B, MM], F32, tag="acc")
                    slots = []
                    for ky in range(KS):
                        for pk in range(5):
                            slots.append((ky, 2 * pk, True))
                        slots.append((ky, 10, False))
                    for si, (ky, kx, paired) in enumerate(slots):
                        kp = 128 if paired else 64
                        rhs = bass.AP(
                            tensor=Xt.tensor,
                            offset=Xt.offset + ky * PADW + kx,
                            ap=[[Xt.ap[0][0], kp], [PADHW, B],
                                [STRIDE * PADW, M6], [STRIDE, M6]])
                        nc.tensor.matmul(acc, lhsT=Wc[0:kp, si * C:(si + 1) * C],
                                         rhs=rhs,
                                         start=(si == 0), stop=(si == len(slots) - 1))
                    nc.scalar.activation(Dt, acc, getattr(AF, _CONV_ACT), bias=Bt)

                # ---- vv^T (+ ones row) per unit ----
                for b in range(B):
                    VT = VTs[b]
                    nc.vector.memset(VT[:, :, 0:1], 1.0)
                    for jt in range(NJT):
                        pt = pp.tile([128, 8], BF, tag="pt", bufs=2)
                        nc.tensor.transpose(pt, VV[:, b, jt * 128:(jt + 1) * 128], ID8)
                        nc.vector.tensor_copy(VT[:, jt, 1:9], pt)

                # ---- per-unit S (rhs, i-side) and R (weights, j-side) ----
                for b in range(B):
                    S, R = Ss[b], Rs[b]
                    nc.sync.dma_start(S[0:8, :].rearrange("p (h w) -> p h w", h=H, w=H),
                                      interior(XQ, b)[0:8])
                    nc.sync.dma_start(R[0:8, :], emb[:, :])
                    # compute engines can't write partition-start 8; build the
                    # broadcast rows at partition 0 and DMA them into rows 8-15.
                    UPQ = stage.tile([8, HW], BF, tag="upq")
                    UPK = stage.tile([8, HW], BF, tag="upk")
                    nc.vector.tensor_scalar_mul(
                        UPQ.rearrange("p (a d) -> p a d", a=MM, d=64),
                        up_ap(QD, b), SCALE)
                    nc.vector.tensor_copy(
                        UPK.rearrange("p (a d) -> p a d", a=MM, d=64), up_ap(KD, b))
                    nc.sync.dma_start(S[8:16, :], UPQ)
                    nc.sync.dma_start(R[8:16, :], UPK)

                if _DEBUG:
                    nc.sync.dma_start(dbg["d_xq"][:, :, :], XQ)
                    nc.sync.dma_start(dbg["d_xk"][:, :, :], XK)
                    nc.sync.dma_start(dbg["d_vv"][:, :, :], VV)
                    nc.sync.dma_start(dbg["d_qd"][:, :, :], QD)
                    nc.sync.dma_start(dbg["d_kd"][:, :, :], KD)
                    nc.sync.dma_start(dbg["d_s0"][:, :], Ss[0])
                    nc.sync.dma_start(dbg["d_r0"][:, :], Rs[0])
                    nc.sync.dma_start(dbg["d_vt0"][:, :],
                                      VTs[0].rearrange("p a b -> p (a b)"))

            # ---- main attention loops ----
            # Flat software pipeline over (b, ichunk, jtile-pair): two
            # E-matmuls fill a 2-bank PSUM tile, ONE 1024-wide exp covers
            # both (amortizing ACT per-op overhead), and the pair's
            # O-matmuls are emitted one step later so PE never stalls on
            # the current exp.
            with tc.tile_pool(name="psum_main", bufs=1, space="PSUM") as pm:
                steps = [(b, i0, ni, jp)
                         for b in range(B)
                         for (i0, ni) in ICHUNKS
                         for jp in range(NJT // 2)]
                po_cur = [None]
                pending = [None]

                def emit_o():
                    pb_, pi0, pni, pjp, pesb = pending[0]
                    if pjp == 0:
                        po_cur[0] = pm.tile([9, 512], F32, tag="po",
                                            bufs=2, name="po")
                    po = po_cur[0]
                    VT = VTs[pb_]
                    nc.tensor.matmul(po[:, :pni], lhsT=VT[:, 2 * pjp, :],
                                     rhs=pesb[:, 0, :pni],
                                     start=(pjp == 0), stop=False)
                    nc.tensor.matmul(po[:, :pni], lhsT=VT[:, 2 * pjp + 1, :],
                                     rhs=pesb[:, 1, :pni],
                                     start=False, stop=(pjp == NJT // 2 - 1))
                    if pjp == NJT // 2 - 1:
                        rec = work.tile([1, 512], F32, tag="rec", name="rec")
                        nc.vector.reciprocal(rec[:, :pni], po[0:1, :pni])
                        pb = pm.tile([9, 512], F32, tag="po", bufs=2, name="pb")
                        nc.tensor.matmul(pb[:, :pni], lhsT=ONE9,
                                         rhs=rec[:, :pni],
                                         start=True, stop=True)
                        pbs = work.tile([9, 512], F32, tag="pbs", name="pbs")
                        nc.vector.tensor_copy(pbs[:, :pni], pb[:, :pni])
                        res = work.tile([9, 512], F32, tag="res", name="res")
                        nc.vector.tensor_mul(res[:, :pni], po[:, :pni],
                                             pbs[:, :pni])
                        nc.sync.dma_start(out[pb_, :, pi0:pi0 + pni],
                                          res[1:9, :pni])

                for step in steps:
                    b, i0, ni, jp = step
                    S, R = Ss[b], Rs[b]
                    pe2 = pm.tile([128, 2, 512], F32, tag="pe", bufs=3,
                                  name="pe2")
                    nc.tensor.matmul(pe2[:, 0, :ni],
                                     lhsT=R[:, (2 * jp) * 128:(2 * jp + 1) * 128],
                                     rhs=S[:, i0:i0 + ni],
                                     start=True, stop=True)
                    nc.tensor.matmul(pe2[:, 1, :ni],
                                     lhsT=R[:, (2 * jp + 1) * 128:(2 * jp + 2) * 128],
                                     rhs=S[:, i0:i0 + ni],
                                     start=True, stop=True)
                    esb2 = epool.tile([128, 2, 512], BF, tag="esb", bufs=6,
                                      name="esb2")
                    nc.scalar.activation(esb2[:, :, :ni], pe2[:, :, :ni], AF.Exp)
                    if pending[0] is not None:
                        emit_o()
                    pending[0] = (b, i0, ni, jp, esb2)
                emit_o()

        for _rep in range(repeat):
            _rep_body()

    if split:
        _split_waits(nc)
    return nc


def _split_waits(nc):
    """This walrus build allows at most ONE sync-wait per instruction.
    Move excess waits onto same-engine NoOps inserted just before."""
    import concourse.mybir as mybir
    ctr = 0
    for fn in nc.m.functions:
        for blk in fn.blocks:
            new = []
            for inst in blk.instructions:
                si = inst.sync_info
                waits = list(si.on_wait) if si and si.on_wait else []
                if len(waits) > 1:
                    for w in waits[:-1]:
                        ctr += 1
                        nop = mybir.InstNoOp(name=f"I-wsplit-{ctr}", ins=[], outs=[])
                        nop.engine = inst.engine
                        nop.sync_info = mybir.SyncInfo(on_wait=[w], on_update=[])
                        new.append(nop)
                    inst.sync_info = mybir.SyncInfo(
                        on_wait=[waits[-1]],
                        on_update=list(si.on_update or []))
                new.append(inst)
            blk.instructions = new


def _get_program(repeat=1):
    if repeat not in _PROGRAMS:
        _PROGRAMS[repeat] = _build_program(repeat)
    return _PROGRAMS[repeat]


def _make_in_maps(f, w_qkv, wq, bq, wk, bk, pos_h, pos_w):
    import ml_dtypes
    BF = ml_dtypes.bfloat16
    f2 = np.ascontiguousarray(f.reshape(B, C, HW)).astype(BF)
    embv = np.ascontiguousarray(
        (pos_h[:, :, None] + pos_w[:, None, :]).reshape(8, HW)).astype(BF)
    w = w_qkv[:, :, 0, 0].astype(np.float32)
    wq = wq.astype(np.float32)
    wk = wk.astype(np.float32)
    in_maps = []
    for h in range(NCORES):
        head = np.arange(h * 8, h * 8 + 8)
        rest = np.delete(np.arange(C), head)
        perm = np.concatenate([head, rest])
        w1T = np.ascontiguousarray(
            np.concatenate([w[0:C][perm].T, w[C:2 * C].T], axis=1)).astype(BF)
        wvT = np.ascontiguousarray(w[2 * C + h * 8: 2 * C + h * 8 + 8].T).astype(BF)
        def pack_taps(wp):
            # [oc', ic', ky, kx] -> [128, 66*64]: 5 (kx,kx+1) pairs + kx=10
            # single per ky row; partner tap weights sit at rows 64-127.
            w2 = np.zeros((128, 66 * C), np.float32)
            si = 0
            for ky in range(KS):
                for pk in range(5):
                    w2[0:64, si * C:(si + 1) * C] = wp[:, :, ky, 2 * pk].T
                    w2[64:128, si * C:(si + 1) * C] = wp[:, :, ky, 2 * pk + 1].T
                    si += 1
                w2[0:64, si * C:(si + 1) * C] = wp[:, :, ky, 10].T
                si += 1
            return np.ascontiguousarray(w2).astype(BF)

        wqp = wq[perm][:, perm]          # [oc', ic', ky, kx]
        wqT = pack_taps(wqp)
        wkp = wk[perm]                   # out-channels permuted, in natural
        wkT = pack_taps(wkp)
        in_maps.append({
            "f2": f2,
            "w1T": w1T,
            "wvT": wvT,
            "wqT": wqT,
            "wkT": wkT,
            "bq2": np.ascontiguousarray(bq[perm].astype(np.float32)),
            "bk2": np.ascontiguousarray(bk[perm].astype(np.float32)),
            "emb": embv,
        })
    return in_maps


def _assemble(results):
    fmap = np.empty((B, C, HW), np.float32)
    for h in range(NCORES):
        fmap[:, h * 8:(h + 1) * 8, :] = results[h]["out"]
    return fmap.reshape(B, C, H, H)


def run(trace=False, **inputs):
    """Run on hardware; returns (output, BassKernelResults)."""
    from concourse.bass_utils import run_bass_kernel_spmd
    nc = _get_program()
    in_maps = _make_in_maps(**inputs)
    res = run_bass_kernel_spmd(nc, in_maps, core_ids=list(range(NCORES)),
                               trace=trace)
    return _assemble(res.results), res


def kernel(**inputs):
    out, _ = run(trace=False, **inputs)
    return out

